# revision 30
# baseline (speedup 1.0000x reference)
# GCNConv (dense adjacency, symmetric normalization) on 8 trn2 NeuronCores.
#
#   out = D^{-1/2} A D^{-1/2} (x @ W) + bias,   deg = A.sum(axis=1)
#
# Row-shard: core c owns output rows [1024c, 1024(c+1)). Its 32MB fp32 shard
# of A streams through SBUF once (2MB HWDGE loads on sync/scalar), cast to
# bf16 (DVE) and kept SBUF-resident (16MB) as AT[j_part=128, cols], where the
# columns are 3 uneven row-chunks of widths [512,384,128] x 64 j-blocks.
#
# deg chunk k (row sums for chunk-k rows) accumulates on PE (ones^T @ AT)
# while the chunk loads; at each chunk end a small AllGather shares it so all
# cores can build dinv[j] for that slice of j-nodes. Chunk widths are chosen
# so AG#k completes just in time: the CC engine is busy until ~55-60us
# (first-collective rendezvous, absorbed by a warm-up AG at t=0) and each AG
# costs ~11us on the single CC stream. The last AG gates only 8 j-blocks.
#
# SpMM outT[d,i] += H'[j,d]^T @ AT[j,i] accumulates into 3 bank-aligned PSUM
# tiles (one per row-chunk; concurrently-open accumulation groups must not
# share a PSUM bank); matmuls are emitted block-by-block as soon as (a) the
# data chunk's cast and (b) the gating AG's dinv are traced, so the PE
# in-order queue never buries ready work behind not-yet-loaded chunks.
#
# Finalize keeps outT[d, i] layout: dinv_loc rows are PE-broadcast to [128,i]
# and applied with one DVE mul (PSUM->SBUF) + per-partition bias add; outT is
# stored contiguous and transposed on the host.

import numpy as np

N = 8192
D = 128
NCORES = 8
P = 128
NB = N // P  # 64 j-blocks
RPC = N // NCORES  # 1024 rows per core
NCH = 3

ICH = [512, 384, 128]  # row-chunk widths
OFF = [0, 512, 896]  # row offsets
S0 = [o // P for o in OFF]  # first 128-sub-block of each chunk
NSB = [w // P for w in ICH]  # sub-blocks per chunk [4,3,1]
ZC = [0]
for w in ICH:
    ZC.append(ZC[-1] + NB * w)  # AT column offset of each chunk
GC = [4096, 3072, 4096]  # load-group widths (2 / 1.5 / 2 MB fp32)
NGR = [NB * ICH[k] // GC[k] for k in range(NCH)]  # groups per chunk [8,8,2]
BPD = [GC[k] // ICH[k] for k in range(NCH)]  # j-blocks per group [8,8,32]


def _u(b):  # which AG unlocks dinv for j-block b
    s = b % 8
    for k in range(NCH):
        if S0[k] <= s < S0[k] + NSB[k]:
            return k
    raise AssertionError


def _col(b):  # column of block b inside dinv_ch[u(b)]
    k = _u(b)
    return (b // 8) * NSB[k] + (b % 8 - S0[k])


def _build():
    from contextlib import ExitStack

    import concourse.bacc as bacc
    import concourse.masks as masks
    import concourse.mybir as mybir
    import concourse.tile as tile

    f32 = mybir.dt.float32
    bf16 = mybir.dt.bfloat16
    mult = mybir.AluOpType.mult
    add = mybir.AluOpType.add

    nc = bacc.Bacc("TRN2", target_bir_lowering=False, debug=False, num_devices=NCORES)

    adjp = nc.dram_tensor("adjp", [P, ZC[NCH]], f32, kind="ExternalInput")
    xT = nc.dram_tensor("xT", [D, N], f32, kind="ExternalInput")
    w = nc.dram_tensor("w", [D, D], f32, kind="ExternalInput")
    bias = nc.dram_tensor("bias", [D], f32, kind="ExternalInput")
    out = nc.dram_tensor("out", [P, RPC], f32, kind="ExternalOutput")  # outT

    with tile.TileContext(nc) as tc, ExitStack() as ctx:
        singles = ctx.enter_context(tc.tile_pool(name="singles", bufs=1))
        dram = ctx.enter_context(tc.tile_pool(name="dram", bufs=1, space="DRAM"))
        atp = ctx.enter_context(tc.tile_pool(name="atp", bufs=1))
        stp = ctx.enter_context(tc.tile_pool(name="stp", bufs=2))
        xcp = ctx.enter_context(tc.tile_pool(name="xcp", bufs=2))
        tmp = ctx.enter_context(tc.tile_pool(name="tmp", bufs=1))
        psdeg = ctx.enter_context(tc.tile_pool(name="psdeg", bufs=1, space="PSUM"))
        psh = ctx.enter_context(tc.tile_pool(name="psh", bufs=1, space="PSUM"))
        psout = ctx.enter_context(tc.tile_pool(name="psout", bufs=1, space="PSUM"))
        psmisc = ctx.enter_context(tc.tile_pool(name="psmisc", bufs=2, space="PSUM"))

        # ---- warm-up AllGather: first thing on gpsimd; the first collective
        # costs ~20-35us (ncfw boot) on top of the cross-core rendezvous
        # barrier (~40-65us launch skew), so burn it on a dummy ----
        wa_in = dram.tile([P], f32, name="wa_in")
        wa_out = dram.tile([NCORES * P], f32, name="wa_out", addr_space="Shared")
        wa_sb = singles.tile([1, P], f32)
        nc.gpsimd.memset(wa_sb[:], 0.0)
        nc.gpsimd.dma_start(wa_in[:], wa_sb[:1, :])
        nc.gpsimd.collective_compute(
            "AllGather",
            mybir.AluOpType.bypass,
            replica_groups=[list(range(NCORES))],
            ins=[wa_in.opt()],
            outs=[wa_out.opt()],
        )

        # ---- constants ----
        ident = singles.tile([P, P], f32)
        masks.make_identity(nc, ident[:])
        ones_bf = singles.tile([P, 1], bf16)
        nc.gpsimd.memset(ones_bf[:], 1.0)
        ones_row = singles.tile([1, P], f32)
        nc.gpsimd.memset(ones_row[:], 1.0)
        w_sb = singles.tile([D, D], f32)
        nc.scalar.dma_start(w_sb[:], w[:, :])
        bias_row = singles.tile([1, D], f32)
        nc.scalar.dma_start(bias_row[:], bias[:])
        bias_ps = psmisc.tile([P, 1], f32, tag="misc")
        nc.tensor.transpose(bias_ps[:, :1], bias_row[:1, :], ident[:1, :1])
        bias_col = singles.tile([P, 1], f32)
        nc.vector.tensor_copy(bias_col[:], bias_ps[:])

        # ---- big SBUF residents ----
        AT = atp.tile([P, ZC[NCH]], bf16)
        Hb = singles.tile([P, NB * D], bf16)  # h, then H' in place

        # bank-aligned accumulators: one PSUM tile per out slice; deg groups
        # may share a bank because they open/close sequentially
        out_ps = [
            psout.tile([P, ICH[k]], f32, name=f"out_ps{k}") for k in range(NCH)
        ]
        deg_a = psdeg.tile([1, 512], f32, name="deg_a")
        deg_b = psdeg.tile([1, 512], f32, name="deg_b")
        deg_ps = [deg_a[:, 0:512], deg_b[:, 0:384], deg_b[:, 384:512]]
        deg_sb = singles.tile([1, RPC], f32)
        dinv_loc = singles.tile([P, RPC], f32)
        out_sb = singles.tile([P, RPC], f32)
        dinv_ch = [
            singles.tile([P, 8 * NSB[k]], f32, name=f"dinv_ch{k}") for k in range(NCH)
        ]
        ag_outs = [None] * NCH

        def rsqrt_newton(dst_ap, ps_ap, width, tag):
            # dst_ap = 1/sqrt(ps_ap), one Newton step, dst doubling as scratch
            sq = tmp.tile([P, width], f32)
            nc.scalar.sqrt(sq[:], ps_ap)
            r0 = tmp.tile([P, width], f32)
            nc.vector.reciprocal(r0[:], sq[:])
            nc.vector.tensor_mul(dst_ap, r0[:], r0[:])
            nc.vector.tensor_mul(dst_ap, dst_ap, ps_ap)
            nc.vector.tensor_scalar(dst_ap, dst_ap, -0.5, 1.5, mult, add)
            nc.vector.tensor_mul(dst_ap, dst_ap, r0[:])

        def ag_chain(k):
            nc.vector.tensor_copy(deg_sb[:, OFF[k] : OFF[k] + ICH[k]], deg_ps[k])
            ag_in = dram.tile([ICH[k]], f32, name=f"ag_in{k}")
            ag_out = dram.tile(
                [NCORES * ICH[k]], f32, name=f"ag_out{k}", addr_space="Shared"
            )
            nc.gpsimd.dma_start(ag_in[:], deg_sb[:1, OFF[k] : OFF[k] + ICH[k]])
            nc.gpsimd.collective_compute(
                "AllGather",
                mybir.AluOpType.bypass,
                replica_groups=[list(range(NCORES))],
                ins=[ag_in.opt()],
                outs=[ag_out.opt()],
            )
            ag_outs[k] = ag_out

        def loc_chain(k):
            # broadcast local deg chunk across partitions, then rsqrt
            bc_ps = psmisc.tile([P, ICH[k]], f32, tag="misc")
            nc.tensor.matmul(
                bc_ps[:], ones_row[:1, :], deg_sb[:1, OFF[k] : OFF[k] + ICH[k]]
            )
            rsqrt_newton(
                dinv_loc[:, OFF[k] : OFF[k] + ICH[k]], bc_ps[:], ICH[k], f"loc{k}"
            )

        def dinv_chain(k):
            # degc load on sync: it waits on AG#k, and by this point the sync
            # queue has issued every load DMA, so a late AG can't stall loads
            nbc = 8 * NSB[k]
            degc = singles.tile([nbc, P], f32, name=f"degc{k}")
            nc.sync.dma_start(degc[:], ag_outs[k][:])
            dgt_ps = psmisc.tile([P, nbc], f32, tag="misc")
            nc.tensor.transpose(dgt_ps[:], degc[:], ident[:nbc, :nbc])
            rsqrt_newton(dinv_ch[k][:], dgt_ps[:], nbc, f"g{k}")

        # ---- SpMM emission machinery ----
        mm_started = [False] * NCH
        mm_count = [0] * NCH
        scaled = [False] * NB

        def emit_mm(b, ic):
            if not scaled[b]:
                # H' scaling on ACT: keeps AG-gated work off the DVE queue,
                # which must keep casting load groups
                k = _u(b)
                nc.scalar.mul(
                    Hb[:, b * D : (b + 1) * D],
                    Hb[:, b * D : (b + 1) * D],
                    dinv_ch[k][:, _col(b) : _col(b) + 1],
                )
                scaled[b] = True
            mm_count[ic] += 1
            nc.tensor.matmul(
                out_ps[ic][:],
                Hb[:, b * D : (b + 1) * D],
                AT[:, ZC[ic] + b * ICH[ic] : ZC[ic] + (b + 1) * ICH[ic]],
                start=not mm_started[ic],
                stop=(mm_count[ic] == NB),
                skip_group_check=True,
            )
            mm_started[ic] = True

        def load_group(k, g, eng, deg=True):
            lo = ZC[k] + g * GC[k]
            stage = stp.tile([P, GC[k]], f32)
            eng.dma_start(stage[:], adjp[:, lo : lo + GC[k]])
            # chunk-2 casts run on the (idle) gpsimd engine: the DVE queue by
            # then holds dinv0's AG-gated Newton ops, which must not gate the
            # last chunk's casts (stage-slot WAR would halt the load)
            ceng = nc.gpsimd if k == 2 else nc.vector
            ceng.tensor_scalar(AT[:, lo : lo + GC[k]], stage[:], 1.0, None, mult)
            if deg:
                deg_mms(k, g)

        def deg_mms(k, g):
            for b in range(g * BPD[k], (g + 1) * BPD[k]):
                nc.tensor.matmul(
                    deg_ps[k],
                    ones_bf[:],
                    AT[:, ZC[k] + b * ICH[k] : ZC[k] + (b + 1) * ICH[k]],
                    start=(b == 0),
                    stop=(b == NB - 1),
                    skip_group_check=True,
                )

        def h_all():
            xch = 1024
            for c0 in range(0, N, xch):
                xc = xcp.tile([D, xch], f32)
                nc.scalar.dma_start(xc[:], xT[:, c0 : c0 + xch])
                for hh in range(xch // 512):
                    h_ps = psh.tile([P, 512], f32)
                    for j in range(4):
                        bb = hh * 4 + j
                        nc.tensor.matmul(
                            h_ps[:, j * P : (j + 1) * P],
                            xc[:, bb * P : (bb + 1) * P],
                            w_sb[:],
                            start=True,
                            stop=True,
                            skip_group_check=True,
                        )
                    b0 = c0 // P + hh * 4
                    nc.vector.tensor_copy(Hb[:, b0 * D : (b0 + 4) * D], h_ps[:])

        # engine assignment for the 18 load groups (sync carries one extra;
        # scalar also carries the 4MB xT + w/bias)
        g_eng = {}
        gidx = 0
        for k in range(NCH):
            for g in range(NGR[k]):
                g_eng[(k, g)] = nc.scalar if gidx % 2 == 1 else nc.sync
                gidx += 1
        g_eng[(2, 1)] = nc.sync  # keep the last group off the busier queue

        U_BLOCKS = [[b for b in range(NB) if _u(b) == k] for k in range(NCH)]

        # ---- schedule ----
        # Invariant: no AG-gated instruction sits in any engine queue ahead
        # of load-critical work (a stalled op blocks everything behind it
        # in-queue and halts the load — observed as a 20us full stall).
        # AG-gated work lives on: ACT after its last DMA issue (scalings),
        # DVE after the last DVE cast (dinv Newtons), sync after its last
        # load DMA (degc gathers), PE after the final deg matmuls (SpMM).
        # chunk 0, then its AG (deg copy ahead of h copies on DVE), then h
        for g in range(NGR[0]):
            load_group(0, g, g_eng[(0, g)])
        ag_chain(0)
        h_all()

        for g in range(NGR[1]):
            load_group(1, g, g_eng[(1, g)])
        ag_chain(1)

        # chunk 2: both DMAs issue (sync) before dinv0's degc; casts on GPS
        load_group(2, 0, g_eng[(2, 0)])
        load_group(2, 1, g_eng[(2, 1)], deg=False)
        dinv_chain(0)
        for b in U_BLOCKS[0]:
            emit_mm(b, 0)
        deg_mms(2, 1)
        ag_chain(2)
        for b in U_BLOCKS[0]:
            emit_mm(b, 1)
        for b in U_BLOCKS[0]:
            emit_mm(b, 2)
        # local dinv chains: all deps met (deg copies done); tail-only engines
        for k in range(NCH):
            loc_chain(k)

        # u1 blocks: all three slices (under AG#2's shadow)
        dinv_chain(1)
        for b in U_BLOCKS[1]:
            emit_mm(b, 0)
            emit_mm(b, 1)
            emit_mm(b, 2)

        # tail: last AG -> dinv2 -> final 24 MMs, finalize slice-by-slice
        dinv_chain(2)
        for ic in range(NCH):
            for b in U_BLOCKS[2]:
                emit_mm(b, ic)
            sl = slice(OFF[ic], OFF[ic] + ICH[ic])
            nc.vector.tensor_mul(out_sb[:, sl], out_ps[ic][:], dinv_loc[:, sl])
            nc.vector.tensor_scalar(out_sb[:, sl], out_sb[:, sl], bias_col[:], None, add)
            nc.sync.dma_start(out[:, sl], out_sb[:, sl])

    nc.compile()
    return nc


_NC_CACHE = {}


def _get_nc():
    if "nc" not in _NC_CACHE:
        _NC_CACHE["nc"] = _build()
    return _NC_CACHE["nc"]


def _pack_shard(adj, c):
    # pack[p, ZC[k] + b*ICH[k] + i] = adj[c*RPC + OFF[k] + i, b*P + p]
    shard = adj[c * RPC : (c + 1) * RPC, :]
    cols = []
    for k in range(NCH):
        t = shard[OFF[k] : OFF[k] + ICH[k], :].T.reshape(NB, P, ICH[k])
        cols.append(t.transpose(1, 0, 2).reshape(P, NB * ICH[k]))
    return np.ascontiguousarray(np.concatenate(cols, axis=1))


def run(x, adj, weight, bias, trace=False):
    from concourse import bass_utils

    x = np.ascontiguousarray(np.asarray(x, dtype=np.float32))
    adj = np.ascontiguousarray(np.asarray(adj, dtype=np.float32))
    weight = np.ascontiguousarray(np.asarray(weight, dtype=np.float32))
    bias = np.ascontiguousarray(np.asarray(bias, dtype=np.float32))

    xTa = np.ascontiguousarray(x.T)
    in_maps = []
    for c in range(NCORES):
        in_maps.append(
            {"adjp": _pack_shard(adj, c), "xT": xTa, "w": weight, "bias": bias}
        )

    nc = _get_nc()
    res = bass_utils.run_bass_kernel_spmd(
        nc, in_maps, core_ids=list(range(NCORES)), trace=trace
    )
    out = np.concatenate([np.ascontiguousarray(r["out"].T) for r in res.results], axis=0)
    return out, res


def kernel(x, adj, weight, bias):
    out, _ = run(x, adj, weight, bias)
    return out


# revision 32
# speedup vs baseline: 1.1527x; 1.1527x over previous
# GCNConv (dense adjacency, symmetric normalization) on 8 trn2 NeuronCores.
#
#   out = D^{-1/2} A D^{-1/2} (x @ W) + bias,   deg = A.sum(axis=1)
#
# Row-shard: core c owns output rows [1024c, 1024(c+1)). Its 32MB fp32 shard
# of A streams through SBUF once (2MB HWDGE loads on sync/scalar), cast to
# bf16 (DVE) and kept SBUF-resident (16MB) as AT[j_part=128, cols], where the
# columns are 3 uneven row-chunks of widths [512,384,128] x 64 j-blocks.
#
# deg chunk k (row sums for chunk-k rows) accumulates on PE (ones^T @ AT)
# while the chunk loads; at each chunk end a small AllGather shares it so all
# cores can build dinv[j] for that slice of j-nodes. Chunk widths are chosen
# so AG#k completes just in time: the CC engine is busy until ~55-60us
# (first-collective rendezvous, absorbed by a warm-up AG at t=0) and each AG
# costs ~11us on the single CC stream. The last AG gates only 8 j-blocks.
#
# SpMM outT[d,i] += H'[j,d]^T @ AT[j,i] accumulates into 3 bank-aligned PSUM
# tiles (one per row-chunk; concurrently-open accumulation groups must not
# share a PSUM bank); matmuls are emitted block-by-block as soon as (a) the
# data chunk's cast and (b) the gating AG's dinv are traced, so the PE
# in-order queue never buries ready work behind not-yet-loaded chunks.
#
# Finalize keeps outT[d, i] layout: dinv_loc rows are PE-broadcast to [128,i]
# and applied with one DVE mul (PSUM->SBUF) + per-partition bias add; outT is
# stored contiguous and transposed on the host.

import numpy as np

N = 8192
D = 128
NCORES = 8
P = 128
NB = N // P  # 64 j-blocks
RPC = N // NCORES  # 1024 rows per core
NCH = 3

ICH = [512, 384, 128]  # row-chunk widths
OFF = [0, 512, 896]  # row offsets
S0 = [o // P for o in OFF]  # first 128-sub-block of each chunk
NSB = [w // P for w in ICH]  # sub-blocks per chunk [4,3,1]
ZC = [0]
for w in ICH:
    ZC.append(ZC[-1] + NB * w)  # AT column offset of each chunk
GC = [4096, 3072, 4096]  # load-group widths (2 / 1.5 / 2 MB fp32)
NGR = [NB * ICH[k] // GC[k] for k in range(NCH)]  # groups per chunk [8,8,2]
BPD = [GC[k] // ICH[k] for k in range(NCH)]  # j-blocks per group [8,8,32]


def _u(b):  # which AG unlocks dinv for j-block b
    s = b % 8
    for k in range(NCH):
        if S0[k] <= s < S0[k] + NSB[k]:
            return k
    raise AssertionError


def _col(b):  # column of block b inside dinv_ch[u(b)]
    k = _u(b)
    return (b // 8) * NSB[k] + (b % 8 - S0[k])


def _build():
    from contextlib import ExitStack

    import concourse.bacc as bacc
    import concourse.masks as masks
    import concourse.mybir as mybir
    import concourse.tile as tile

    f32 = mybir.dt.float32
    bf16 = mybir.dt.bfloat16
    mult = mybir.AluOpType.mult
    add = mybir.AluOpType.add

    nc = bacc.Bacc("TRN2", target_bir_lowering=False, debug=False, num_devices=NCORES)

    adjp = nc.dram_tensor("adjp", [P, ZC[NCH]], f32, kind="ExternalInput")
    xT = nc.dram_tensor("xT", [D, N], f32, kind="ExternalInput")
    w = nc.dram_tensor("w", [D, D], f32, kind="ExternalInput")
    bias = nc.dram_tensor("bias", [D], f32, kind="ExternalInput")
    out = nc.dram_tensor("out", [P, RPC], f32, kind="ExternalOutput")  # outT

    with tile.TileContext(nc) as tc, ExitStack() as ctx:
        singles = ctx.enter_context(tc.tile_pool(name="singles", bufs=1))
        dram = ctx.enter_context(tc.tile_pool(name="dram", bufs=1, space="DRAM"))
        atp = ctx.enter_context(tc.tile_pool(name="atp", bufs=1))
        stp = ctx.enter_context(tc.tile_pool(name="stp", bufs=2))
        xcp = ctx.enter_context(tc.tile_pool(name="xcp", bufs=2))
        tmp = ctx.enter_context(tc.tile_pool(name="tmp", bufs=1))
        psdeg = ctx.enter_context(tc.tile_pool(name="psdeg", bufs=1, space="PSUM"))
        psh = ctx.enter_context(tc.tile_pool(name="psh", bufs=1, space="PSUM"))
        psout = ctx.enter_context(tc.tile_pool(name="psout", bufs=1, space="PSUM"))
        psmisc = ctx.enter_context(tc.tile_pool(name="psmisc", bufs=2, space="PSUM"))

        # ---- warm-up AllGather: first thing on gpsimd; the first collective
        # costs ~20-35us (ncfw boot) on top of the cross-core rendezvous
        # barrier (~40-65us launch skew), so burn it on a dummy ----
        wa_in = dram.tile([P], f32, name="wa_in")
        wa_out = dram.tile([NCORES * P], f32, name="wa_out", addr_space="Shared")
        wa_sb = singles.tile([1, P], f32)
        nc.gpsimd.memset(wa_sb[:], 0.0)
        nc.gpsimd.dma_start(wa_in[:], wa_sb[:1, :])
        nc.gpsimd.collective_compute(
            "AllGather",
            mybir.AluOpType.bypass,
            replica_groups=[list(range(NCORES))],
            ins=[wa_in.opt()],
            outs=[wa_out.opt()],
        )

        # ---- constants ----
        ident = singles.tile([P, P], f32)
        masks.make_identity(nc, ident[:])
        ones_bf = singles.tile([P, 1], bf16)
        nc.gpsimd.memset(ones_bf[:], 1.0)
        ones_row = singles.tile([1, P], f32)
        nc.gpsimd.memset(ones_row[:], 1.0)
        w_sb = singles.tile([D, D], f32)
        nc.scalar.dma_start(w_sb[:], w[:, :])
        bias_row = singles.tile([1, D], f32)
        nc.scalar.dma_start(bias_row[:], bias[:])
        bias_ps = psmisc.tile([P, 1], f32, tag="misc")
        nc.tensor.transpose(bias_ps[:, :1], bias_row[:1, :], ident[:1, :1])
        bias_col = singles.tile([P, 1], f32)
        nc.vector.tensor_copy(bias_col[:], bias_ps[:])

        # ---- big SBUF residents ----
        AT = atp.tile([P, ZC[NCH]], bf16)
        Hb = singles.tile([P, NB * D], bf16)  # h, then H' in place

        # bank-aligned accumulators: one PSUM tile per out slice; deg groups
        # may share a bank because they open/close sequentially
        out_ps = [
            psout.tile([P, ICH[k]], f32, name=f"out_ps{k}") for k in range(NCH)
        ]
        deg_a = psdeg.tile([1, 512], f32, name="deg_a")
        deg_b = psdeg.tile([1, 512], f32, name="deg_b")
        deg_ps = [deg_a[:, 0:512], deg_b[:, 0:384], deg_b[:, 384:512]]
        deg_sb = singles.tile([1, RPC], f32)
        dinv_loc = singles.tile([P, RPC], f32)
        out_sb = singles.tile([P, RPC], f32)
        dinv_ch = [
            singles.tile([P, 8 * NSB[k]], f32, name=f"dinv_ch{k}") for k in range(NCH)
        ]
        ag_outs = [None] * NCH

        def rsqrt_newton(dst_ap, ps_ap, width, tag):
            # dst_ap = 1/sqrt(ps_ap), one Newton step, dst doubling as scratch
            sq = tmp.tile([P, width], f32)
            nc.scalar.sqrt(sq[:], ps_ap)
            r0 = tmp.tile([P, width], f32)
            nc.vector.reciprocal(r0[:], sq[:])
            nc.vector.tensor_mul(dst_ap, r0[:], r0[:])
            nc.vector.tensor_mul(dst_ap, dst_ap, ps_ap)
            nc.vector.tensor_scalar(dst_ap, dst_ap, -0.5, 1.5, mult, add)
            nc.vector.tensor_mul(dst_ap, dst_ap, r0[:])

        def ag_chain(k):
            nc.vector.tensor_copy(deg_sb[:, OFF[k] : OFF[k] + ICH[k]], deg_ps[k])
            ag_in = dram.tile([ICH[k]], f32, name=f"ag_in{k}")
            ag_out = dram.tile(
                [NCORES * ICH[k]], f32, name=f"ag_out{k}", addr_space="Shared"
            )
            nc.gpsimd.dma_start(ag_in[:], deg_sb[:1, OFF[k] : OFF[k] + ICH[k]])
            nc.gpsimd.collective_compute(
                "AllGather",
                mybir.AluOpType.bypass,
                replica_groups=[list(range(NCORES))],
                ins=[ag_in.opt()],
                outs=[ag_out.opt()],
            )
            ag_outs[k] = ag_out

        def loc_chain(k):
            # broadcast local deg chunk across partitions, then rsqrt
            bc_ps = psmisc.tile([P, ICH[k]], f32, tag="misc")
            nc.tensor.matmul(
                bc_ps[:], ones_row[:1, :], deg_sb[:1, OFF[k] : OFF[k] + ICH[k]]
            )
            rsqrt_newton(
                dinv_loc[:, OFF[k] : OFF[k] + ICH[k]], bc_ps[:], ICH[k], f"loc{k}"
            )

        def dinv_chain(k):
            # degc load on sync: it waits on AG#k, and by this point the sync
            # queue has issued every load DMA, so a late AG can't stall loads
            nbc = 8 * NSB[k]
            degc = singles.tile([nbc, P], f32, name=f"degc{k}")
            nc.sync.dma_start(degc[:], ag_outs[k][:])
            dgt_ps = psmisc.tile([P, nbc], f32, tag="misc")
            nc.tensor.transpose(dgt_ps[:], degc[:], ident[:nbc, :nbc])
            rsqrt_newton(dinv_ch[k][:], dgt_ps[:], nbc, f"g{k}")

        # ---- SpMM emission machinery ----
        mm_started = [False] * NCH
        mm_count = [0] * NCH
        scaled = [False] * NB

        def emit_mm(b, ic):
            if not scaled[b]:
                # H' scaling on ACT: keeps AG-gated work off the DVE queue,
                # which must keep casting load groups
                k = _u(b)
                nc.scalar.mul(
                    Hb[:, b * D : (b + 1) * D],
                    Hb[:, b * D : (b + 1) * D],
                    dinv_ch[k][:, _col(b) : _col(b) + 1],
                )
                scaled[b] = True
            mm_count[ic] += 1
            nc.tensor.matmul(
                out_ps[ic][:],
                Hb[:, b * D : (b + 1) * D],
                AT[:, ZC[ic] + b * ICH[ic] : ZC[ic] + (b + 1) * ICH[ic]],
                start=not mm_started[ic],
                stop=(mm_count[ic] == NB),
                skip_group_check=True,
            )
            mm_started[ic] = True

        def load_group(k, g, eng, deg=True):
            lo = ZC[k] + g * GC[k]
            stage = stp.tile([P, GC[k]], f32)
            eng.dma_start(stage[:], adjp[:, lo : lo + GC[k]])
            nc.vector.tensor_scalar(AT[:, lo : lo + GC[k]], stage[:], 1.0, None, mult)
            if deg:
                deg_mms(k, g)

        def deg_mms(k, g):
            for b in range(g * BPD[k], (g + 1) * BPD[k]):
                nc.tensor.matmul(
                    deg_ps[k],
                    ones_bf[:],
                    AT[:, ZC[k] + b * ICH[k] : ZC[k] + (b + 1) * ICH[k]],
                    start=(b == 0),
                    stop=(b == NB - 1),
                    skip_group_check=True,
                )

        def h_all():
            xch = 1024
            for c0 in range(0, N, xch):
                xc = xcp.tile([D, xch], f32)
                nc.scalar.dma_start(xc[:], xT[:, c0 : c0 + xch])
                for hh in range(xch // 512):
                    h_ps = psh.tile([P, 512], f32)
                    for j in range(4):
                        bb = hh * 4 + j
                        nc.tensor.matmul(
                            h_ps[:, j * P : (j + 1) * P],
                            xc[:, bb * P : (bb + 1) * P],
                            w_sb[:],
                            start=True,
                            stop=True,
                            skip_group_check=True,
                        )
                    b0 = c0 // P + hh * 4
                    nc.vector.tensor_copy(Hb[:, b0 * D : (b0 + 4) * D], h_ps[:])

        # engine assignment for the 18 load groups (sync carries one extra;
        # scalar also carries the 4MB xT + w/bias)
        g_eng = {}
        gidx = 0
        for k in range(NCH):
            for g in range(NGR[k]):
                g_eng[(k, g)] = nc.scalar if gidx % 2 == 1 else nc.sync
                gidx += 1
        g_eng[(2, 1)] = nc.sync  # keep the last group off the busier queue

        U_BLOCKS = [[b for b in range(NB) if _u(b) == k] for k in range(NCH)]

        # ---- schedule ----
        # Invariant: no AG-gated instruction sits in any engine queue ahead
        # of load-critical work (a stalled op blocks everything behind it
        # in-queue and halts the load — observed as a 20us full stall).
        # AG-gated work lives on: ACT after its last DMA issue (scalings),
        # DVE after the last DVE cast (dinv Newtons), sync after its last
        # load DMA (degc gathers), PE after the final deg matmuls (SpMM).
        # chunk 0, then its AG (deg copy ahead of h copies on DVE), then h
        for g in range(NGR[0]):
            load_group(0, g, g_eng[(0, g)])
        ag_chain(0)
        h_all()

        for g in range(NGR[1]):
            load_group(1, g, g_eng[(1, g)])
        ag_chain(1)

        # chunk 2, fully load-critical order; all AG-gated work after it
        load_group(2, 0, g_eng[(2, 0)])
        load_group(2, 1, g_eng[(2, 1)])
        ag_chain(2)
        # dinv0's DVE Newton now sits after every cast/deg-copy on DVE: a
        # late AG#0 can no longer stall the load
        dinv_chain(0)
        for b in U_BLOCKS[0]:
            emit_mm(b, 0)
            emit_mm(b, 1)
            emit_mm(b, 2)
        # local dinv chains: all deps met (deg copies done); tail-only engines
        for k in range(NCH):
            loc_chain(k)

        # u1 blocks: all three slices (under AG#2's shadow)
        dinv_chain(1)
        for b in U_BLOCKS[1]:
            emit_mm(b, 0)
            emit_mm(b, 1)
            emit_mm(b, 2)

        # tail: last AG -> dinv2 -> final 24 MMs, finalize slice-by-slice
        dinv_chain(2)
        for ic in range(NCH):
            for b in U_BLOCKS[2]:
                emit_mm(b, ic)
            sl = slice(OFF[ic], OFF[ic] + ICH[ic])
            nc.vector.tensor_mul(out_sb[:, sl], out_ps[ic][:], dinv_loc[:, sl])
            nc.vector.tensor_scalar(out_sb[:, sl], out_sb[:, sl], bias_col[:], None, add)
            nc.sync.dma_start(out[:, sl], out_sb[:, sl])

    nc.compile()
    return nc


_NC_CACHE = {}


def _get_nc():
    if "nc" not in _NC_CACHE:
        _NC_CACHE["nc"] = _build()
    return _NC_CACHE["nc"]


def _pack_shard(adj, c):
    # pack[p, ZC[k] + b*ICH[k] + i] = adj[c*RPC + OFF[k] + i, b*P + p]
    shard = adj[c * RPC : (c + 1) * RPC, :]
    cols = []
    for k in range(NCH):
        t = shard[OFF[k] : OFF[k] + ICH[k], :].T.reshape(NB, P, ICH[k])
        cols.append(t.transpose(1, 0, 2).reshape(P, NB * ICH[k]))
    return np.ascontiguousarray(np.concatenate(cols, axis=1))


def run(x, adj, weight, bias, trace=False):
    from concourse import bass_utils

    x = np.ascontiguousarray(np.asarray(x, dtype=np.float32))
    adj = np.ascontiguousarray(np.asarray(adj, dtype=np.float32))
    weight = np.ascontiguousarray(np.asarray(weight, dtype=np.float32))
    bias = np.ascontiguousarray(np.asarray(bias, dtype=np.float32))

    xTa = np.ascontiguousarray(x.T)
    in_maps = []
    for c in range(NCORES):
        in_maps.append(
            {"adjp": _pack_shard(adj, c), "xT": xTa, "w": weight, "bias": bias}
        )

    nc = _get_nc()
    res = bass_utils.run_bass_kernel_spmd(
        nc, in_maps, core_ids=list(range(NCORES)), trace=trace
    )
    out = np.concatenate([np.ascontiguousarray(r["out"].T) for r in res.results], axis=0)
    return out, res


def kernel(x, adj, weight, bias):
    out, _ = run(x, adj, weight, bias)
    return out
